# revision 10
# baseline (speedup 1.0000x reference)
"""Beam-search decoder (nn_BeamDecoder) as a Trainium2 Bass kernel.

Strategy: vocab-shard W_out across the 8 NeuronCores (4000 columns each, held
resident in SBUF).  Every core computes logits for all B*BEAM rows over its
vocab shard each step, takes a per-row top-8 (values+indices) plus a partial
sum-of-exp, and an AllGather exchanges those per-shard candidates.  Every core
then redundantly computes the global top-4 per batch element (exact beam
update), gathers the winners' token embeddings with an indirect DMA, and
proceeds to the next step.  Token sequences are reconstructed on the host from
the per-step backpointers (standard beam-search backtracking — bitwise
equivalent to materializing the reordered token buffer).
"""

import numpy as np

B, BEAM, D, V, S = 16, 4, 1024, 32000, 128
NCORES = 8
VS = V // NCORES          # 4000 vocab columns per core
CW = 500                  # psum chunk width
NCH = VS // CW            # 8 chunks per step
KC = D // 128             # 8 contraction chunks
ROWS = B * BEAM           # 64
NEG = -1.0e9

_BUILD_CACHE = {}


def _ensure_paths():
    import sys
    if "/opt/trn_rl_repo" not in sys.path:
        sys.path.insert(0, "/opt/trn_rl_repo")


def build_bass(nsteps, dbg_stop=None):
    """Build + compile the SPMD Bass program for `nsteps` device steps.

    dbg_stop: debug bisection point — one of None (full), "pre", "gather",
    "mm", "pay", "cc". When set, the program is truncated after that stage
    (outputs may be garbage; only used to localize hangs).
    """
    _ensure_paths()
    if nsteps in _BUILD_CACHE and dbg_stop is None:
        return _BUILD_CACHE[nsteps]

    import concourse.bacc as bacc
    import concourse.mybir as mybir
    import concourse.tile as tile
    from concourse import bass

    f32, i32, u32 = mybir.dt.float32, mybir.dt.int32, mybir.dt.uint32
    EXP = mybir.ActivationFunctionType.Exp
    LN = mybir.ActivationFunctionType.Ln
    ADD = mybir.AluOpType.add
    MULT = mybir.AluOpType.mult
    EQ = mybir.AluOpType.is_equal

    nc = bacc.Bacc("TRN2", target_bir_lowering=False, debug=False,
                   num_devices=NCORES)

    enc_d = nc.dram_tensor("enc", [B, S, D], f32, kind="ExternalInput")
    mask_d = nc.dram_tensor("mask", [B, S], f32, kind="ExternalInput")
    first_d = nc.dram_tensor("first", [B], i32, kind="ExternalInput")
    emb_d = nc.dram_tensor("emb", [V, D], f32, kind="ExternalInput")
    w_d = nc.dram_tensor("w", [D, VS], f32, kind="ExternalInput")

    outw_d = nc.dram_tensor("out_words", [max(nsteps, 1), B, BEAM], i32,
                            kind="ExternalOutput")
    outj_d = nc.dram_tensor("out_j", [max(nsteps, 1), B, BEAM], u32,
                            kind="ExternalOutput")
    outs_d = nc.dram_tensor("out_scores", [B, BEAM], f32, kind="ExternalOutput")

    ag_ins = [nc.dram_tensor(f"ag_in_{i}", [ROWS, 9], f32, kind="Internal")
              for i in range(nsteps)]
    ag_outs = [nc.dram_tensor(f"ag_out_{i}", [NCORES, ROWS, 9], f32,
                              kind="Internal", addr_space="Shared")
               for i in range(nsteps)]

    # ---- constants (embedded in the NEFF) ----
    ident_d = nc.inline_tensor(np.eye(128, dtype=np.float32), name="c_ident")
    sone_np = np.zeros((B, ROWS), np.float32)
    sone_np[np.arange(ROWS) // BEAM, np.arange(ROWS)] = 1.0
    sone_d = nc.inline_tensor(sone_np, name="c_sone")
    # candidate j = k*32 + s*4 + c ; global word = local_idx + s*VS
    offs_np = np.tile(((np.arange(128) % 32) // 4 * VS).astype(np.float32), (B, 1))
    offs_d = nc.inline_tensor(offs_np, name="c_offs")
    iota128_d = nc.inline_tensor(
        np.tile(np.arange(128, dtype=np.float32), (B, 1)), name="c_iota128")
    iota64_d = nc.inline_tensor(
        np.tile(np.arange(64, dtype=np.float32), (ROWS, 1)), name="c_iota64")

    with tile.TileContext(nc) as tc:
        with (
            tc.tile_pool(name="persist", bufs=1) as per,
            tc.tile_pool(name="work", bufs=2) as wk,
            tc.tile_pool(name="psum", bufs=2, space="PSUM") as pp,
        ):
            # ---------- persistent tiles ----------
            ident = per.tile([128, 128], f32, tag="ident")
            nc.sync.dma_start(out=ident[:], in_=ident_d.ap())
            sone = per.tile([B, ROWS], f32, tag="sone")
            nc.sync.dma_start(out=sone[:], in_=sone_d.ap())
            offs = per.tile([B, 128], f32, tag="offs")
            nc.sync.dma_start(out=offs[:], in_=offs_d.ap())
            iota128 = per.tile([B, 128], f32, tag="iota128")
            nc.sync.dma_start(out=iota128[:], in_=iota128_d.ap())
            iota64 = per.tile([ROWS, 64], f32, tag="iota64")
            nc.sync.dma_start(out=iota64[:], in_=iota64_d.ap())

            w_sb = []
            for kc in range(KC):
                t = per.tile([128, VS], f32, tag=f"w{kc}")
                nc.sync.dma_start(out=t[:], in_=w_d[kc * 128:(kc + 1) * 128, :])
                w_sb.append(t)

            enc_log = per.tile([B, VS], f32, tag="enc_log")
            scores = per.tile([B, BEAM], f32, tag="scores")
            idx64 = per.tile([ROWS, 1], i32, tag="idx64")

            # ---------- preamble: enc_pool and ENC_LOG ----------
            mask_sb = per.tile([B, S], f32, tag="mask")
            nc.sync.dma_start(out=mask_sb[:], in_=mask_d.ap())
            msum = per.tile([B, 1], f32, tag="msum")
            nc.vector.reduce_sum(out=msum[:], in_=mask_sb[:],
                                 axis=mybir.AxisListType.X)
            nc.vector.tensor_scalar_add(msum[:], msum[:], 1e-6)
            minv = per.tile([B, 1], f32, tag="minv")
            nc.vector.reciprocal(minv[:], msum[:])

            maskT = per.tile([S, B], f32, tag="maskT")
            nc.sync.dma_start(out=maskT[:], in_=mask_d.ap().rearrange("b s -> s b"))

            # enc_poolT_raw[d, b] = sum_s enc[b, s, d] * mask[b, s]
            poolp = pp.tile([128, KC * B], f32, space="PSUM", tag="lpa")
            poolp3 = poolp[:].rearrange("p (kc b) -> p kc b", kc=KC)
            for b in range(B):
                encb = wk.tile([S, D], f32, tag="encb")
                nc.sync.dma_start(out=encb[:], in_=enc_d[b])
                for kc in range(KC):
                    nc.tensor.matmul(
                        out=poolp3[:, kc, b:b + 1],
                        lhsT=encb[:, kc * 128:(kc + 1) * 128],
                        rhs=maskT[:, b:b + 1],
                        start=True, stop=True, skip_group_check=True)
            encT = per.tile([128, KC * B], f32, tag="encT")
            nc.vector.tensor_copy(out=encT[:], in_=poolp[:])
            encT3 = encT[:].rearrange("p (kc b) -> p kc b", kc=KC)

            # ENC_LOG = (enc_poolT_raw.T @ W) * minv   [B, VS]
            for vc in range(NCH):
                elp = pp.tile([B, CW], f32, space="PSUM", tag="lpb")
                for kc in range(KC):
                    nc.tensor.matmul(
                        out=elp[:], lhsT=encT3[:, kc, :],
                        rhs=w_sb[kc][:, vc * CW:(vc + 1) * CW],
                        start=(kc == 0), stop=(kc == KC - 1))
                nc.vector.tensor_scalar(
                    out=enc_log[:, vc * CW:(vc + 1) * CW], in0=elp[:],
                    scalar1=minv[:, 0:1], scalar2=None, op0=MULT)

            # ---------- state init ----------
            nc.vector.memset(scores[:, 0:1], 0.0)
            nc.vector.memset(scores[:, 1:BEAM], NEG)
            nc.sync.dma_start(
                out=idx64[:],
                in_=first_d.ap()[:, None].to_broadcast([B, BEAM]))

            # ---------- decode steps ----------
            for i in range(1, (0 if dbg_stop == "pre" else nsteps) + 1):
                embt = wk.tile([ROWS, D], f32, tag="embt")
                nc.gpsimd.indirect_dma_start(
                    out=embt[:], out_offset=None, in_=emb_d.ap(),
                    in_offset=bass.IndirectOffsetOnAxis(ap=idx64[:, :1], axis=0))

                embT = wk.tile([128, KC * ROWS], f32, tag="embT")
                embT3 = embT[:].rearrange("p (kc r) -> p kc r", kc=KC)
                for kc in range(KC):
                    trp = pp.tile([128, ROWS], f32, space="PSUM", tag="trp")
                    nc.tensor.transpose(
                        out=trp[:], in_=embt[:, kc * 128:(kc + 1) * 128],
                        identity=ident[:ROWS, :ROWS])
                    nc.vector.tensor_copy(out=embT3[:, kc, :], in_=trp[:])

                if dbg_stop == "gather":
                    continue
                expf = wk.tile([ROWS, VS], f32, tag="expf")
                sump = wk.tile([ROWS, NCH], f32, tag="sump")
                vals64 = wk.tile([ROWS, 64], f32, tag="vals64")
                idxf64 = wk.tile([ROWS, 64], f32, tag="idxf64")

                # logits chunk-pairs: one weight load serves two 500-col streams
                for t in range(NCH // 2):
                    ca, cb = 2 * t, 2 * t + 1
                    lpa = pp.tile([ROWS, CW], f32, space="PSUM", tag="lpa")
                    lpb = pp.tile([ROWS, CW], f32, space="PSUM", tag="lpb")
                    for kc in range(KC):
                        nc.tensor.matmul(
                            out=lpa[:], lhsT=embT3[:, kc, :],
                            rhs=w_sb[kc][:, ca * CW:(ca + 1) * CW],
                            start=(kc == 0), stop=False)
                        nc.tensor.matmul(
                            out=lpb[:], lhsT=embT3[:, kc, :],
                            rhs=w_sb[kc][:, cb * CW:(cb + 1) * CW],
                            start=(kc == 0), stop=False)
                    nc.tensor.matmul(
                        out=lpa[:], lhsT=sone[:],
                        rhs=enc_log[:, ca * CW:(ca + 1) * CW],
                        start=False, stop=True)
                    nc.tensor.matmul(
                        out=lpb[:], lhsT=sone[:],
                        rhs=enc_log[:, cb * CW:(cb + 1) * CW],
                        start=False, stop=True)

                    for c, lp in ((ca, lpa), (cb, lpb)):
                        sl = slice(c * CW, (c + 1) * CW)
                        nc.scalar.activation(
                            out=expf[:, sl], in_=lp[:], func=EXP,
                            accum_out=sump[:, c:c + 1])
                        # per-chunk top-8 (values + local indices)
                        nc.vector.max(out=vals64[:, c * 8:(c + 1) * 8],
                                      in_=expf[:, sl])
                        ci8 = wk.tile([ROWS, 8], u32, tag="ci8")
                        nc.vector.max_index(
                            out=ci8[:], in_max=vals64[:, c * 8:(c + 1) * 8],
                            in_values=expf[:, sl])
                        cif = wk.tile([ROWS, 8], f32, tag="cif")
                        nc.vector.tensor_copy(out=cif[:], in_=ci8[:])
                        nc.vector.tensor_scalar_add(
                            idxf64[:, c * 8:(c + 1) * 8], cif[:], float(c * CW))

                if dbg_stop == "mm":
                    continue
                # per-row top-8 across the 64 chunk-candidates
                fmax8 = wk.tile([ROWS, 8], f32, tag="fmax8")
                fpos8 = wk.tile([ROWS, 8], u32, tag="fpos8")
                nc.vector.max(out=fmax8[:], in_=vals64[:])
                nc.vector.max_index(out=fpos8[:], in_max=fmax8[:], in_values=vals64[:])
                posf = wk.tile([ROWS, 4], f32, tag="posf")
                nc.vector.tensor_copy(out=posf[:], in_=fpos8[:, 0:4])
                if dbg_stop == "pay1":
                    continue

                pay = wk.tile([ROWS, 9], f32, tag="pay")
                nc.scalar.activation(out=pay[:, 0:4], in_=fmax8[:, 0:4], func=LN)
                if dbg_stop == "pay2":
                    continue
                eq64 = wk.tile([ROWS, 64], f32, tag="eq64")
                scr64 = wk.tile([ROWS, 64], f32, tag="scr64")
                for s4 in range(4):
                    nc.vector.tensor_scalar(
                        out=eq64[:], in0=iota64[:], scalar1=posf[:, s4:s4 + 1],
                        scalar2=None, op0=EQ)
                    nc.vector.tensor_tensor(
                        out=scr64[:], in0=eq64[:], in1=idxf64[:], op=MULT)
                    nc.vector.reduce_sum(out=pay[:, 4 + s4:5 + s4],
                                         in_=scr64[:], axis=mybir.AxisListType.X)
                if dbg_stop == "pay3":
                    continue
                nc.vector.reduce_sum(out=pay[:, 8:9], in_=sump[:],
                                     axis=mybir.AxisListType.X)

                if dbg_stop == "pay":
                    continue
                # ---- exchange per-shard candidates ----
                nc.sync.dma_start(out=ag_ins[i - 1].ap(), in_=pay[:])
                nc.gpsimd.collective_compute(
                    "AllGather", mybir.AluOpType.bypass,
                    replica_groups=[list(range(NCORES))],
                    ins=[ag_ins[i - 1].ap()], outs=[ag_outs[i - 1].ap()])

                comb = wk.tile([B, BEAM * NCORES * 9], f32, tag="comb")
                comb4 = comb[:].rearrange("b (k s w) -> b k s w", k=BEAM, s=NCORES)
                nc.sync.dma_start(
                    out=comb4,
                    in_=ag_outs[i - 1].ap().rearrange(
                        "s (b k) w -> b k s w", b=B, k=BEAM))

                if dbg_stop == "cc":
                    continue
                # ---- global beam update (identical on every core) ----
                gsum = wk.tile([B, BEAM], f32, tag="gsum")
                nc.vector.reduce_sum(out=gsum[:], in_=comb4[:, :, :, 8:9],
                                     axis=mybir.AxisListType.XY)
                lse = wk.tile([B, BEAM], f32, tag="lse")
                nc.scalar.activation(out=lse[:], in_=gsum[:], func=LN)
                adj = wk.tile([B, BEAM], f32, tag="adj")
                nc.vector.tensor_sub(adj[:], scores[:], lse[:])

                cand = wk.tile([B, 128], f32, tag="cand")
                cand4 = cand[:].rearrange("b (k s c) -> b k s c", k=BEAM, s=NCORES)
                nc.vector.tensor_tensor(
                    out=cand4, in0=comb4[:, :, :, 0:4],
                    in1=adj[:].to_broadcast([B, BEAM, NCORES, 4]),
                    op=ADD)
                candw = wk.tile([B, 128], f32, tag="candw")
                candw4 = candw[:].rearrange("b (k s c) -> b k s c", k=BEAM, s=NCORES)
                nc.vector.tensor_tensor(
                    out=candw4, in0=comb4[:, :, :, 4:8],
                    in1=offs[:].rearrange("b (k s c) -> b k s c", k=BEAM, s=NCORES),
                    op=ADD)

                win8 = wk.tile([B, 8], f32, tag="win8")
                winj8 = wk.tile([B, 8], u32, tag="winj8")
                nc.vector.max(out=win8[:], in_=cand[:])
                nc.vector.max_index(out=winj8[:], in_max=win8[:], in_values=cand[:])
                nc.vector.tensor_copy(out=scores[:], in_=win8[:, 0:4])

                jf = wk.tile([B, 4], f32, tag="jf")
                nc.vector.tensor_copy(out=jf[:], in_=winj8[:, 0:4])
                words_f = wk.tile([B, BEAM], f32, tag="words_f")
                eqb = wk.tile([B, 128], f32, tag="eqb")
                scrb = wk.tile([B, 128], f32, tag="scrb")
                for s4 in range(4):
                    nc.vector.tensor_scalar(
                        out=eqb[:], in0=iota128[:], scalar1=jf[:, s4:s4 + 1],
                        scalar2=None, op0=EQ)
                    nc.vector.tensor_tensor(
                        out=scrb[:], in0=eqb[:], in1=candw[:], op=MULT)
                    nc.vector.reduce_sum(out=words_f[:, s4:s4 + 1],
                                         in_=scrb[:], axis=mybir.AxisListType.X)
                words_i = wk.tile([B, BEAM], i32, tag="words_i")
                nc.vector.tensor_copy(out=words_i[:], in_=words_f[:])

                nc.sync.dma_start(out=outw_d[i - 1], in_=words_i[:])
                nc.sync.dma_start(out=outj_d[i - 1], in_=winj8[:, 0:4])
                # winner words become next step's gather indices [64, 1]
                nc.sync.dma_start(out=idx64[:], in_=words_i[:])

            nc.sync.dma_start(out=outs_d.ap(), in_=scores[:])

    nc.compile()
    _BUILD_CACHE[nsteps] = nc
    return nc


def make_in_maps(encoder_states, src_mask, tgt_first, token_emb, W_out):
    enc = np.ascontiguousarray(np.asarray(encoder_states, dtype=np.float32))
    mask = np.ascontiguousarray(np.asarray(src_mask, dtype=np.float32))
    first = np.ascontiguousarray(np.asarray(tgt_first, dtype=np.int32).reshape(B))
    emb = np.ascontiguousarray(np.asarray(token_emb, dtype=np.float32))
    w = np.asarray(W_out, dtype=np.float32)
    base = {"enc": enc, "mask": mask, "first": first, "emb": emb}
    return [dict(base, w=np.ascontiguousarray(w[:, c * VS:(c + 1) * VS]))
            for c in range(NCORES)]


def decode_outputs(out_words, out_j, out_scores, tgt_first, max_steps):
    nsteps = max_steps - 1
    tokens = np.zeros((B, max_steps), np.int32)
    tokens[:, 0] = np.asarray(tgt_first, dtype=np.int32).reshape(B)
    words = np.asarray(out_words).astype(np.int64)
    jarr = np.asarray(out_j).astype(np.int64)
    for b in range(B):
        k = 0
        for i in range(nsteps, 0, -1):
            tokens[b, i] = words[i - 1, b, k]
            k = jarr[i - 1, b, k] // 32
    scores = np.asarray(out_scores, dtype=np.float32).reshape(B, BEAM)
    return tokens, scores


def kernel(encoder_states, src_mask, tgt_first, token_emb, W_out, max_steps):
    _ensure_paths()
    max_steps = int(max_steps)
    nsteps = max_steps - 1
    if nsteps <= 0:
        tokens = np.zeros((B, max_steps), np.int32)
        tokens[:, 0] = np.asarray(tgt_first, dtype=np.int32).reshape(B)
        scores = np.full((B, BEAM), np.float32(NEG), dtype=np.float32)
        scores[:, 0] = 0.0
        return tokens, scores

    from concourse import bass_utils

    nc = build_bass(nsteps)
    in_maps = make_in_maps(encoder_states, src_mask, tgt_first, token_emb, W_out)
    res = bass_utils.run_bass_kernel_spmd(nc, in_maps,
                                          core_ids=list(range(NCORES)))
    r0 = res.results[0]
    return decode_outputs(r0["out_words"], r0["out_j"], r0["out_scores"],
                          tgt_first, max_steps)


# revision 20
# speedup vs baseline: 1.2221x; 1.2221x over previous
"""Beam-search decoder (nn_BeamDecoder) as a Trainium2 Bass kernel.

Strategy: vocab-shard W_out across the 8 NeuronCores (4000 columns each, held
resident in SBUF).  Every core computes logits for all B*BEAM rows over its
vocab shard each step, takes a per-row top-8 (values+indices) plus a partial
sum-of-exp, and an AllGather exchanges those per-shard candidates.  Every core
then redundantly computes the global top-4 per batch element (exact beam
update), gathers the winners' token embeddings with an indirect DMA, and
proceeds to the next step.  Token sequences are reconstructed on the host from
the per-step backpointers (standard beam-search backtracking — bitwise
equivalent to materializing the reordered token buffer).
"""

import numpy as np

B, BEAM, D, V, S = 16, 4, 1024, 32000, 128
NCORES = 8
VS = V // NCORES          # 4000 vocab columns per core
CW = 500                  # psum chunk width
NCH = VS // CW            # 8 chunks per step
KC = D // 128             # 8 contraction chunks
ROWS = B * BEAM           # 64
NEG = -1.0e9

_BUILD_CACHE = {}


def _ensure_paths():
    import sys
    if "/opt/trn_rl_repo" not in sys.path:
        sys.path.insert(0, "/opt/trn_rl_repo")


def build_bass(nsteps, dbg_stop=None):
    """Build + compile the SPMD Bass program for `nsteps` device steps.

    dbg_stop: debug bisection point — one of None (full), "pre", "gather",
    "mm", "pay", "cc". When set, the program is truncated after that stage
    (outputs may be garbage; only used to localize hangs).
    """
    _ensure_paths()
    if nsteps in _BUILD_CACHE and dbg_stop is None:
        return _BUILD_CACHE[nsteps]

    import concourse.bacc as bacc
    import concourse.mybir as mybir
    import concourse.tile as tile
    from concourse import bass

    f32, i32, u32 = mybir.dt.float32, mybir.dt.int32, mybir.dt.uint32
    f32r = mybir.dt.float32r
    f16 = mybir.dt.float16
    EXP = mybir.ActivationFunctionType.Exp
    LN = mybir.ActivationFunctionType.Ln
    ADD = mybir.AluOpType.add
    MULT = mybir.AluOpType.mult
    EQ = mybir.AluOpType.is_equal

    nc = bacc.Bacc("TRN2", target_bir_lowering=False, debug=False,
                   num_devices=NCORES)

    enc_d = nc.dram_tensor("enc", [B, S, D], f32, kind="ExternalInput")
    mask_d = nc.dram_tensor("mask", [B, S], f32, kind="ExternalInput")
    first_d = nc.dram_tensor("first", [B], i32, kind="ExternalInput")
    emb_d = nc.dram_tensor("emb", [V, D], f32, kind="ExternalInput")
    # W shard pre-scaled by 64 and split into fp16 hi+lo on the host:
    # hi = fp16(64*W), lo = fp16(64*W - hi).  Together ~22 mantissa bits.
    whi_d = nc.dram_tensor("w_hi", [D, VS], f16, kind="ExternalInput")
    wlo_d = nc.dram_tensor("w_lo", [D, VS], f16, kind="ExternalInput")

    outw_d = nc.dram_tensor("out_words", [max(nsteps, 1), B, BEAM], i32,
                            kind="ExternalOutput")
    outj_d = nc.dram_tensor("out_j", [max(nsteps, 1), B, BEAM], u32,
                            kind="ExternalOutput")
    outs_d = nc.dram_tensor("out_scores", [B, BEAM], f32, kind="ExternalOutput")

    ag_ins = [nc.dram_tensor(f"ag_in_{i}", [ROWS, 9], f32, kind="Internal")
              for i in range(nsteps)]
    ag_outs = [nc.dram_tensor(f"ag_out_{i}", [NCORES, ROWS, 9], f32,
                              kind="Internal", addr_space="Shared")
               for i in range(nsteps)]

    # ---- constants (embedded in the NEFF) ----
    ident_d = nc.inline_tensor(np.eye(128, dtype=np.float32), name="c_ident")
    sone_np = np.zeros((B, ROWS), np.float32)
    sone_np[np.arange(ROWS) // BEAM, np.arange(ROWS)] = 1.0
    sone_d = nc.inline_tensor(sone_np, name="c_sone")
    # candidate j = k*32 + s*4 + c ; global word = local_idx + s*VS
    offs_np = np.tile(((np.arange(128) % 32) // 4 * VS).astype(np.float32), (B, 1))
    offs_d = nc.inline_tensor(offs_np, name="c_offs")
    iota128_d = nc.inline_tensor(
        np.tile(np.arange(128, dtype=np.float32), (B, 1)), name="c_iota128")
    iota64_d = nc.inline_tensor(
        np.tile(np.arange(64, dtype=np.float32), (ROWS, 1)), name="c_iota64")

    with tile.TileContext(nc) as tc:
        with (
            tc.tile_pool(name="persist", bufs=1) as per,
            tc.tile_pool(name="work", bufs=2) as wk,
            tc.tile_pool(name="psum", bufs=2, space="PSUM") as pp,
        ):
            # ---------- persistent tiles ----------
            ident = per.tile([128, 128], f32, tag="ident")
            nc.sync.dma_start(out=ident[:], in_=ident_d.ap())
            sone = per.tile([B, ROWS], f32, tag="sone")
            nc.sync.dma_start(out=sone[:], in_=sone_d.ap())
            offs = per.tile([B, 128], f32, tag="offs")
            nc.sync.dma_start(out=offs[:], in_=offs_d.ap())
            iota128 = per.tile([B, 128], f32, tag="iota128")
            nc.sync.dma_start(out=iota128[:], in_=iota128_d.ap())
            iota64 = per.tile([ROWS, 64], f32, tag="iota64")
            nc.sync.dma_start(out=iota64[:], in_=iota64_d.ap())

            whi_sb, wlo_sb = [], []
            for kc in range(KC):
                t = per.tile([128, VS], f16, tag=f"whi{kc}")
                nc.sync.dma_start(out=t[:], in_=whi_d[kc * 128:(kc + 1) * 128, :])
                whi_sb.append(t)
                t = per.tile([128, VS], f16, tag=f"wlo{kc}")
                nc.sync.dma_start(out=t[:], in_=wlo_d[kc * 128:(kc + 1) * 128, :])
                wlo_sb.append(t)

            ep64 = per.tile([ROWS, D], f32, tag="ep64")
            scores = per.tile([B, BEAM], f32, tag="scores")
            idx64 = per.tile([ROWS, 1], i32, tag="idx64")

            # ---------- preamble: enc_pool and ENC_LOG ----------
            mask_sb = per.tile([B, S], f32, tag="mask")
            nc.sync.dma_start(out=mask_sb[:], in_=mask_d.ap())
            msum = per.tile([B, 1], f32, tag="msum")
            nc.vector.reduce_sum(out=msum[:], in_=mask_sb[:],
                                 axis=mybir.AxisListType.X)
            nc.vector.tensor_scalar_add(msum[:], msum[:], 1e-6)
            minv = per.tile([B, 1], f32, tag="minv")
            nc.vector.reciprocal(minv[:], msum[:])

            maskT = per.tile([S, B], f32, tag="maskT")
            nc.sync.dma_start(out=maskT[:], in_=mask_d.ap().rearrange("b s -> s b"))

            # enc_poolT_raw[d, b] = sum_s enc[b, s, d] * mask[b, s]
            poolp = pp.tile([128, KC * B], f32, space="PSUM", tag="lpa")
            poolp3 = poolp[:].rearrange("p (kc b) -> p kc b", kc=KC)
            for b in range(B):
                encb = wk.tile([S, D], f32, tag="encb")
                nc.sync.dma_start(out=encb[:], in_=enc_d[b])
                for kc in range(KC):
                    nc.tensor.matmul(
                        out=poolp3[:, kc, b:b + 1],
                        lhsT=encb[:, kc * 128:(kc + 1) * 128],
                        rhs=maskT[:, b:b + 1],
                        start=True, stop=True, skip_group_check=True)
            encT = per.tile([128, KC * B], f32, tag="encT")
            nc.vector.tensor_copy(out=encT[:], in_=poolp[:])
            encT3 = encT[:].rearrange("p (kc b) -> p kc b", kc=KC)

            # enc_pool rows [B, D] = (enc_poolT_raw).T * minv
            ep16 = per.tile([B, D], f32, tag="ep16")
            for kc in range(KC):
                tp = pp.tile([B, 128], f32, space="PSUM", tag="lpb")
                nc.tensor.transpose(out=tp[:], in_=encT3[:, kc, :],
                                    identity=ident[:, :])
                nc.vector.tensor_scalar(
                    out=ep16[:, kc * 128:(kc + 1) * 128], in0=tp[:],
                    scalar1=minv[:, 0:1], scalar2=None, op0=MULT)
            # expand per-batch pool rows to the 64 beam rows: ep64 = sone.T @ ep16
            for half in range(2):
                sl = slice(half * 512, (half + 1) * 512)
                e64p = pp.tile([ROWS, 512], f32, space="PSUM", tag="lpa")
                nc.tensor.matmul(out=e64p[:], lhsT=sone[:], rhs=ep16[:, sl],
                                 start=True, stop=True, skip_group_check=True)
                nc.vector.tensor_copy(out=ep64[:, sl], in_=e64p[:])

            # ---------- state init ----------
            nc.vector.memset(scores[:, 0:1], 0.0)
            nc.vector.memset(scores[:, 1:BEAM], NEG)
            nc.sync.dma_start(
                out=idx64[:],
                in_=first_d.ap()[:, None].to_broadcast([B, BEAM]))

            # ---------- decode steps ----------
            for i in range(1, (0 if dbg_stop == "pre" else nsteps) + 1):
                embt = wk.tile([ROWS, D], f32, tag="embt")
                nc.gpsimd.indirect_dma_start(
                    out=embt[:], out_offset=None, in_=emb_d.ap(),
                    in_offset=bass.IndirectOffsetOnAxis(ap=idx64[:, :1], axis=0))

                # h = token_emb[word] + enc_pool  (beam-row space)
                nc.vector.tensor_tensor(out=embt[:], in0=embt[:], in1=ep64[:],
                                        op=ADD)

                embT = wk.tile([128, KC * ROWS], f32, tag="embT")
                embT3 = embT[:].rearrange("p (kc r) -> p kc r", kc=KC)
                for kc in range(KC):
                    trp = pp.tile([128, ROWS], f32, space="PSUM", tag="trp")
                    nc.tensor.transpose(
                        out=trp[:], in_=embt[:, kc * 128:(kc + 1) * 128],
                        identity=ident[:ROWS, :ROWS])
                    nc.vector.tensor_copy(out=embT3[:, kc, :], in_=trp[:])
                # scale by 64 and split into fp16 hi + lo
                h64 = wk.tile([128, KC * ROWS], f32, tag="h64")
                nc.vector.tensor_scalar(out=h64[:], in0=embT[:], scalar1=64.0,
                                        scalar2=None, op0=MULT)
                hiT = wk.tile([128, KC * ROWS], f16, tag="hiT")
                nc.vector.tensor_copy(out=hiT[:], in_=h64[:])
                loT = wk.tile([128, KC * ROWS], f16, tag="loT")
                nc.vector.tensor_tensor(out=loT[:], in0=h64[:], in1=hiT[:],
                                        op=mybir.AluOpType.subtract)
                hiT3 = hiT[:].rearrange("p (kc r) -> p kc r", kc=KC)
                loT3 = loT[:].rearrange("p (kc r) -> p kc r", kc=KC)

                if dbg_stop == "gather":
                    continue
                expf = wk.tile([ROWS, VS], f32, tag="expf")
                sump = wk.tile([ROWS, NCH], f32, tag="sump")
                vals64 = wk.tile([ROWS, 64], f32, tag="vals64")
                idxf64 = wk.tile([ROWS, 64], f32, tag="idxf64")

                # logits chunk-pairs: 3 fp16 terms; one weight load serves
                # several 500-col streams
                for t in range(NCH // 2):
                    ca, cb = 2 * t, 2 * t + 1
                    lpa = pp.tile([ROWS, CW], f32, space="PSUM", tag="lpa")
                    lpb = pp.tile([ROWS, CW], f32, space="PSUM", tag="lpb")
                    sla = slice(ca * CW, (ca + 1) * CW)
                    slb = slice(cb * CW, (cb + 1) * CW)
                    for kc in range(KC):
                        last = kc == KC - 1
                        nc.tensor.matmul(
                            out=lpa[:], lhsT=hiT3[:, kc, :],
                            rhs=whi_sb[kc][:, sla],
                            start=(kc == 0), stop=False)
                        nc.tensor.matmul(
                            out=lpb[:], lhsT=hiT3[:, kc, :],
                            rhs=whi_sb[kc][:, slb],
                            start=(kc == 0), stop=False)
                        nc.tensor.matmul(
                            out=lpa[:], lhsT=hiT3[:, kc, :],
                            rhs=wlo_sb[kc][:, sla], start=False, stop=False)
                        nc.tensor.matmul(
                            out=lpb[:], lhsT=hiT3[:, kc, :],
                            rhs=wlo_sb[kc][:, slb], start=False, stop=False)
                        nc.tensor.matmul(
                            out=lpa[:], lhsT=loT3[:, kc, :],
                            rhs=whi_sb[kc][:, sla], start=False, stop=last)
                        nc.tensor.matmul(
                            out=lpb[:], lhsT=loT3[:, kc, :],
                            rhs=whi_sb[kc][:, slb], start=False, stop=last)

                    for c, lp in ((ca, lpa), (cb, lpb)):
                        sl = slice(c * CW, (c + 1) * CW)
                        nc.scalar.activation(
                            out=expf[:, sl], in_=lp[:], func=EXP,
                            scale=1.0 / 4096.0,
                            accum_out=sump[:, c:c + 1])
                        # per-chunk top-8 (values + local indices)
                        nc.vector.max(out=vals64[:, c * 8:(c + 1) * 8],
                                      in_=expf[:, sl])
                        ci8 = wk.tile([ROWS, 8], u32, tag="ci8")
                        nc.vector.max_index(
                            out=ci8[:], in_max=vals64[:, c * 8:(c + 1) * 8],
                            in_values=expf[:, sl])
                        cif = wk.tile([ROWS, 8], f32, tag="cif")
                        nc.vector.tensor_copy(out=cif[:], in_=ci8[:])
                        nc.vector.tensor_scalar_add(
                            idxf64[:, c * 8:(c + 1) * 8], cif[:], float(c * CW))

                if dbg_stop == "mm":
                    continue
                # per-row top-8 across the 64 chunk-candidates
                fmax8 = wk.tile([ROWS, 8], f32, tag="fmax8")
                fpos8 = wk.tile([ROWS, 8], u32, tag="fpos8")
                nc.vector.max(out=fmax8[:], in_=vals64[:])
                nc.vector.max_index(out=fpos8[:], in_max=fmax8[:], in_values=vals64[:])
                posf = wk.tile([ROWS, 4], f32, tag="posf")
                nc.vector.tensor_copy(out=posf[:], in_=fpos8[:, 0:4])
                if dbg_stop == "pay1":
                    continue

                pay = wk.tile([ROWS, 9], f32, tag="pay")
                nc.scalar.activation(out=pay[:, 0:4], in_=fmax8[:, 0:4], func=LN)
                if dbg_stop == "pay2":
                    continue
                eq64 = wk.tile([ROWS, 64], f32, tag="eq64")
                scr64 = wk.tile([ROWS, 64], f32, tag="scr64")
                for s4 in range(4):
                    nc.vector.tensor_scalar(
                        out=eq64[:], in0=iota64[:], scalar1=posf[:, s4:s4 + 1],
                        scalar2=None, op0=EQ)
                    nc.vector.tensor_tensor(
                        out=scr64[:], in0=eq64[:], in1=idxf64[:], op=MULT)
                    nc.vector.reduce_sum(out=pay[:, 4 + s4:5 + s4],
                                         in_=scr64[:], axis=mybir.AxisListType.X)
                if dbg_stop == "pay3":
                    continue
                nc.vector.reduce_sum(out=pay[:, 8:9], in_=sump[:],
                                     axis=mybir.AxisListType.X)

                if dbg_stop == "pay":
                    continue
                # ---- exchange per-shard candidates ----
                nc.sync.dma_start(out=ag_ins[i - 1].ap(), in_=pay[:])
                nc.gpsimd.collective_compute(
                    "AllGather", mybir.AluOpType.bypass,
                    replica_groups=[list(range(NCORES))],
                    ins=[ag_ins[i - 1].ap()], outs=[ag_outs[i - 1].ap()])

                comb = wk.tile([B, BEAM * NCORES * 9], f32, tag="comb")
                comb4 = comb[:].rearrange("b (k s w) -> b k s w", k=BEAM, s=NCORES)
                nc.sync.dma_start(
                    out=comb4,
                    in_=ag_outs[i - 1].ap().rearrange(
                        "s (b k) w -> b k s w", b=B, k=BEAM))

                if dbg_stop == "cc":
                    continue
                # ---- global beam update (identical on every core) ----
                gsum = wk.tile([B, BEAM], f32, tag="gsum")
                nc.vector.reduce_sum(out=gsum[:], in_=comb4[:, :, :, 8:9],
                                     axis=mybir.AxisListType.XY)
                lse = wk.tile([B, BEAM], f32, tag="lse")
                nc.scalar.activation(out=lse[:], in_=gsum[:], func=LN)
                adj = wk.tile([B, BEAM], f32, tag="adj")
                nc.vector.tensor_sub(adj[:], scores[:], lse[:])

                cand = wk.tile([B, 128], f32, tag="cand")
                cand4 = cand[:].rearrange("b (k s c) -> b k s c", k=BEAM, s=NCORES)
                nc.vector.tensor_tensor(
                    out=cand4, in0=comb4[:, :, :, 0:4],
                    in1=adj[:].to_broadcast([B, BEAM, NCORES, 4]),
                    op=ADD)
                candw = wk.tile([B, 128], f32, tag="candw")
                candw4 = candw[:].rearrange("b (k s c) -> b k s c", k=BEAM, s=NCORES)
                nc.vector.tensor_tensor(
                    out=candw4, in0=comb4[:, :, :, 4:8],
                    in1=offs[:].rearrange("b (k s c) -> b k s c", k=BEAM, s=NCORES),
                    op=ADD)

                win8 = wk.tile([B, 8], f32, tag="win8")
                winj8 = wk.tile([B, 8], u32, tag="winj8")
                nc.vector.max(out=win8[:], in_=cand[:])
                nc.vector.max_index(out=winj8[:], in_max=win8[:], in_values=cand[:])
                nc.vector.tensor_copy(out=scores[:], in_=win8[:, 0:4])

                jf = wk.tile([B, 4], f32, tag="jf")
                nc.vector.tensor_copy(out=jf[:], in_=winj8[:, 0:4])
                words_f = wk.tile([B, BEAM], f32, tag="words_f")
                eqb = wk.tile([B, 128], f32, tag="eqb")
                scrb = wk.tile([B, 128], f32, tag="scrb")
                for s4 in range(4):
                    nc.vector.tensor_scalar(
                        out=eqb[:], in0=iota128[:], scalar1=jf[:, s4:s4 + 1],
                        scalar2=None, op0=EQ)
                    nc.vector.tensor_tensor(
                        out=scrb[:], in0=eqb[:], in1=candw[:], op=MULT)
                    nc.vector.reduce_sum(out=words_f[:, s4:s4 + 1],
                                         in_=scrb[:], axis=mybir.AxisListType.X)
                words_i = wk.tile([B, BEAM], i32, tag="words_i")
                nc.vector.tensor_copy(out=words_i[:], in_=words_f[:])

                nc.sync.dma_start(out=outw_d[i - 1], in_=words_i[:])
                nc.sync.dma_start(out=outj_d[i - 1], in_=winj8[:, 0:4])
                # winner words become next step's gather indices [64, 1]
                nc.sync.dma_start(out=idx64[:], in_=words_i[:])

            nc.sync.dma_start(out=outs_d.ap(), in_=scores[:])

    nc.compile()
    _BUILD_CACHE[nsteps] = nc
    return nc


def make_in_maps(encoder_states, src_mask, tgt_first, token_emb, W_out):
    enc = np.ascontiguousarray(np.asarray(encoder_states, dtype=np.float32))
    mask = np.ascontiguousarray(np.asarray(src_mask, dtype=np.float32))
    first = np.ascontiguousarray(np.asarray(tgt_first, dtype=np.int32).reshape(B))
    emb = np.ascontiguousarray(np.asarray(token_emb, dtype=np.float32))
    w = np.asarray(W_out, dtype=np.float32)
    base = {"enc": enc, "mask": mask, "first": first, "emb": emb}
    maps = []
    for c in range(NCORES):
        w64 = np.ascontiguousarray(w[:, c * VS:(c + 1) * VS]) * np.float32(64.0)
        w_hi = w64.astype(np.float16)
        w_lo = (w64 - w_hi.astype(np.float32)).astype(np.float16)
        maps.append(dict(base, w_hi=w_hi, w_lo=w_lo))
    return maps


def decode_outputs(out_words, out_j, out_scores, tgt_first, max_steps):
    nsteps = max_steps - 1
    tokens = np.zeros((B, max_steps), np.int32)
    tokens[:, 0] = np.asarray(tgt_first, dtype=np.int32).reshape(B)
    words = np.asarray(out_words).astype(np.int64)
    jarr = np.asarray(out_j).astype(np.int64)
    for b in range(B):
        k = 0
        for i in range(nsteps, 0, -1):
            tokens[b, i] = words[i - 1, b, k]
            k = jarr[i - 1, b, k] // 32
    scores = np.asarray(out_scores, dtype=np.float32).reshape(B, BEAM)
    return tokens, scores


def kernel(encoder_states, src_mask, tgt_first, token_emb, W_out, max_steps):
    _ensure_paths()
    max_steps = int(max_steps)
    nsteps = max_steps - 1
    if nsteps <= 0:
        tokens = np.zeros((B, max_steps), np.int32)
        tokens[:, 0] = np.asarray(tgt_first, dtype=np.int32).reshape(B)
        scores = np.full((B, BEAM), np.float32(NEG), dtype=np.float32)
        scores[:, 0] = 0.0
        return tokens, scores

    from concourse import bass_utils

    nc = build_bass(nsteps)
    in_maps = make_in_maps(encoder_states, src_mask, tgt_first, token_emb, W_out)
    res = bass_utils.run_bass_kernel_spmd(nc, in_maps,
                                          core_ids=list(range(NCORES)))
    r0 = res.results[0]
    return decode_outputs(r0["out_words"], r0["out_j"], r0["out_scores"],
                          tgt_first, max_steps)
